# revision 33
# baseline (speedup 1.0000x reference)
"""MatchingNet forward on 8 Trainium2 NeuronCores (Bass/Tile), v2.

Math (reference):
    s_emb = l2norm(support @ W + b)   [Ns, E]
    q_emb = l2norm(query @ W + b)     [Nq, E]
    sims  = q_emb @ s_emb.T           [Nq, Ns]
    preds = softmax(sims, axis=1) @ one_hot(labels, C)   [Nq, C]

Sharding: query rows data-parallel (1024/core); support encode sharded
(512 rows/core), normalized embeddings AllGathered in 4 x 128-row
chunks so the first chunk's sims start as early as possible.

Layout: embeddings are computed TRANSPOSED ([emb, n], emb on
partitions), so the chain needs no transposes. preds are accumulated
TRANSPOSED ([cls, q]) with one_hot as the stationary operand; the
softmax denominator is the row-sum of the class numerators (one_hot
rows sum to 1), and the final [C, Nq] output is transposed on host.

Scheduling: support encode runs FIRST and ships immediately (the
collectives' wall-clock floor is the NRT CC-core boot barrier, but
early doorbells keep us off the critical path). Query encode is
m-outer so each weight pair loads once and streams both 512-query
blocks. preds matmuls trail the sims/exp chain by two support chunks
so the PE never waits on the ACT exp.
"""

import numpy as np
import ml_dtypes

import concourse.bacc as bacc
import concourse.mybir as mybir
import concourse.tile as tile
from concourse.bass_utils import run_bass_kernel_spmd

F32 = mybir.dt.float32
BF16 = mybir.dt.bfloat16
FP8 = mybir.dt.float8e4
# normalized embeddings scaled by 16 before the fp8 cast; the sims matmul
# result is scaled back inside exp(). W scaled by 32 (fp8 range); the
# encoder bias-add scales back.
EMB_SCALE = 16.0
W_SCALE = 32.0
AF = mybir.ActivationFunctionType
DR = mybir.MatmulPerfMode.DoubleRow

N_SUPPORT = 4096
N_QUERY = 8192
IN_DIM = 2048
EMB_DIM = 1024
N_CLS = 64
N_CORES = 8
NQ_SHARD = N_QUERY // N_CORES  # 1024 query rows per core
G_CHUNKS = 2                   # AllGather chunks per core (256 rows each)


def build_nc(NS, NQ, IN, EMB, NCLS, n_cores=N_CORES):
    KCH = IN // 128    # contraction chunks for the encoder matmul
    MCH = EMB // 128   # emb chunks (partition blocks of the embT layout)
    NS_SH = NS // n_cores
    NB_Q = NQ // 512
    G = G_CHUNKS
    CW = NS_SH // G    # rows per gather chunk
    SCH = NS // 128    # global 128-row support chunks
    assert NS_SH == 512 and NQ % 512 == 0 and KCH % 2 == 0 and MCH % 2 == 0
    assert CW % 128 == 0

    nc = bacc.Bacc()
    supX = nc.declare_dram_parameter("supX", [128, KCH, 512], FP8,
                                     isOutput=False)
    qX = nc.declare_dram_parameter("qX", [NB_Q, 128, KCH, 512], FP8,
                                   isOutput=False)
    Wd = nc.declare_dram_parameter("W", [MCH, 128, KCH, 128], FP8,
                                   isOutput=False)
    bd = nc.declare_dram_parameter("b", [128, MCH], F32, isOutput=False)
    ohd = nc.declare_dram_parameter("onehot", [128, SCH, NCLS], BF16,
                                    isOutput=False)
    # transposed output [C, NQ]; host transposes back
    outd = nc.declare_dram_parameter("out", [NCLS, NQ], F32, isOutput=True)

    with tile.TileContext(nc) as tc:
        with (
            tc.tile_pool(name="singles", bufs=1) as singles,
            tc.tile_pool(name="emb_pool", bufs=1) as emb_pool,
            tc.tile_pool(name="small", bufs=4) as small,
            tc.tile_pool(name="ps_mm", bufs=6, space="PSUM") as ps_mm,
        ):
            b_sb = singles.tile([128, MCH], F32)
            nc.sync.dma_start(out=b_sb, in_=bd[:, :])
            b16_sb = singles.tile([128, MCH], F32)
            nc.scalar.activation(b16_sb, b_sb, AF.Identity, scale=EMB_SCALE)
            neg64 = singles.tile([128, 1], F32)
            nc.vector.memset(neg64, -64.0)
            b2to26 = singles.tile([128, 1], F32)
            nc.vector.memset(b2to26, float(1024 * 65536))
            # fp8 ones pair for DoubleRow partition-reduction matmuls
            ones2 = singles.tile([128, 2, 16], FP8)
            nc.vector.memset(ones2, 1.0)
            oh_sb = singles.tile([128, SCH, NCLS], BF16)
            nc.scalar.dma_start(out=oh_sb, in_=ohd[:, :, :])

            q_nrm = emb_pool.tile([128, MCH, NQ], FP8, name="q_nrm")
            # UN-normalized support embeddings (x@W+b)*16, drained straight
            # from PSUM by a second ACT pass; each row's 1/||.|| rides along
            # as fp8 hi/lo deltas of n2 and is applied inside the receiver's
            # exp() via a per-partition scale AP.
            s_ship = emb_pool.tile([128, MCH, 512], FP8, name="s_ship")
            SUB = CW // 128
            gt = [[emb_pool.tile([128, MCH, CW], FP8, name=f"gt{g}_{c}",
                                 tag=f"gt{g}_{c}")
                   for c in range(n_cores)] for g in range(G)]
            gt_n2 = [[emb_pool.tile([128, 2 * SUB], FP8, name=f"gn{g}_{c}",
                                    tag=f"gn{g}_{c}")
                      for c in range(n_cores)] for g in range(G)]

            with (
                tc.tile_pool(name="w_pool", bufs=1) as w_pool,
                tc.tile_pool(name="xin", bufs=3) as xin,
                tc.tile_pool(name="pre_pool", bufs=3) as pre_pool,
                tc.tile_pool(name="sq_pool", bufs=3) as sq_pool,
                tc.tile_pool(name="ps_n2", bufs=2, space="PSUM") as ps_n2,
                tc.tile_pool(name="bc_pool", bufs=2) as bc_pool,
                tc.tile_pool(name="dscr", bufs=2, space="DRAM") as dscr,
                tc.tile_pool(name="cc_pool", bufs=1, space="DRAM") as cc_pool,
            ):
                # --- input loads ---
                # sync queue: supX in k-pair slices (the first m-group
                # streams pairs as they land); scalar queue: W0 pairs (small,
                # lands fast) then the query blocks.
                W_sb = [w_pool.tile([128, KCH, 128], FP8, tag=f"w{m}",
                                    name=f"w{m}") for m in range(MCH)]
                sup_xk = xin.tile([128, KCH, 512], FP8, tag="sxk", name="sxk")
                for t in range(KCH // 2):
                    nc.scalar.dma_start(out=W_sb[0][:, 2 * t:2 * t + 2, :],
                                        in_=Wd[0][:, 2 * t:2 * t + 2, :])
                for q4 in range(4):
                    kq = KCH // 4
                    nc.sync.dma_start(
                        out=sup_xk[:, q4 * kq:(q4 + 1) * kq, :],
                        in_=supX[:, q4 * kq:(q4 + 1) * kq, :])
                for m in range(1, MCH):
                    nc.sync.dma_start(out=W_sb[m], in_=Wd[m])
                q_xks = []
                for nb in range(NB_Q):
                    t_ = xin.tile([128, KCH, 512], FP8, tag="qxk", name="qxk")
                    nc.scalar.dma_start(out=t_, in_=qX[nb])
                    q_xks.append(t_)

                def norm_chain(n2_ps, pre, dst, cols):
                    """broadcast n2, rsqrt on 128 partitions, apply.

                    Single-partition [1, N] DVE/ACT ops serialize over N
                    (3.3us for a reciprocal), so broadcast FIRST (via DRAM;
                    zero partition step is only legal on DRAM sources) and
                    run one [128, N] Rsqrt instead."""
                    n2_sb = small.tile([1, cols], F32, tag="n2sb",
                                       name="n2sb")
                    nc.scalar.activation(n2_sb, n2_ps, AF.Identity)
                    iscr = dscr.tile([1, cols], F32, tag="iscr", name="iscr")
                    nc.sync.dma_start(out=iscr, in_=n2_sb)
                    n2b = bc_pool.tile([128, cols], F32, tag="n2b",
                                       name="n2b")
                    nc.sync.dma_start(out=n2b,
                                      in_=iscr.partition_broadcast(128))
                    nrmb = bc_pool.tile([128, cols], F32, tag="nrmb",
                                        name="nrmb")
                    nc.scalar.activation(nrmb, n2b, AF.Sqrt,
                                         scale=1.0 / (EMB_SCALE * EMB_SCALE))
                    invb = bc_pool.tile([128, cols], F32, tag="invb",
                                        name="invb")
                    nc.vector.reciprocal(invb, nrmb)
                    for m in range(MCH):
                        nc.vector.tensor_mul(dst[m], pre[:, m, :], invb)

                # --- support encode: one 512-col block, ship ASAP ---
                pre_s = pre_pool.tile([128, MCH, 512], BF16, tag="pre",
                                      name="pre_s")
                sq_s = sq_pool.tile([128, MCH, 512], FP8, tag="sq",
                                    name="sq_s")
                n2_s = ps_n2.tile([1, 512], F32, tag="n2", name="n2s")
                for m in range(MCH):
                    ps = ps_mm.tile([128, 512], F32, tag="mmps", name="ps")
                    for t in range(KCH // 2):
                        nc.tensor.matmul(
                            ps, lhsT=W_sb[m][:, 2 * t:2 * t + 2, :],
                            rhs=sup_xk[:, 2 * t:2 * t + 2, :],
                            start=(t == 0), stop=(t == KCH // 2 - 1),
                            perf_mode=DR)
                    nc.scalar.activation(pre_s[:, m, :], ps, AF.Identity,
                                         bias=b_sb[:, m:m + 1],
                                         scale=1.0 / W_SCALE)
                    # second drain: the shipped fp8 payload (16x scale)
                    nc.scalar.activation(s_ship[:, m, :], ps, AF.Identity,
                                         bias=b16_sb[:, m:m + 1],
                                         scale=EMB_SCALE / W_SCALE)
                    nc.vector.tensor_mul(sq_s[:, m, :], pre_s[:, m, :],
                                         pre_s[:, m, :])
                    # trailing DoubleRow ones-matmul partition reduction
                    if m >= 2 and m % 2 == 0:
                        p = m // 2 - 1
                        nc.tensor.matmul(
                            n2_s, lhsT=ones2[:, :, 0:1],
                            rhs=sq_s[:, 2 * p:2 * p + 2, :],
                            start=(p == 0), stop=False, perf_mode=DR)
                p = MCH // 2 - 1
                nc.tensor.matmul(n2_s, lhsT=ones2[:, :, 0:1],
                                 rhs=sq_s[:, 2 * p:2 * p + 2, :],
                                 start=False, stop=True, perf_mode=DR)

                def ship_support():
                    """pack n2 as fp8 hi/lo deltas and AllGather the
                    un-normalized payload in G chunks.

                    n2 ~ 1024 +- ~200; d = (n2-1024)/16 fits fp8 with 6%
                    relative error of the DEVIATION, and lo = d - fp8(d)
                    recovers it to ~0.04% of n2."""
                    n2sb = small.tile([1, 512], F32, tag="n2sb", name="n2sb")
                    nc.scalar.activation(n2sb, n2_s, AF.Identity)
                    iscr = dscr.tile([1, 512], F32, tag="iscr", name="iscr")
                    nc.sync.dma_start(out=iscr, in_=n2sb)
                    n2p = small.tile([128, 4], F32, tag="n2p", name="n2p")
                    nc.sync.dma_start(
                        out=n2p,
                        in_=iscr.rearrange("o (j p) -> (o p) j", p=128))
                    d32 = small.tile([128, 4], F32, tag="d32", name="d32")
                    nc.scalar.activation(d32, n2p, AF.Identity,
                                         scale=1.0 / 16.0, bias=neg64)
                    hi8 = small.tile([128, 4], FP8, tag="hi8", name="hi8")
                    nc.scalar.activation(hi8, n2p, AF.Identity,
                                         scale=1.0 / 16.0, bias=neg64)
                    lo8 = small.tile([128, 4], FP8, tag="lo8", name="lo8")
                    nc.vector.tensor_sub(lo8, d32, hi8)
                    outs = []
                    PW = MCH * CW
                    for g in range(G):
                        ag_in = cc_pool.tile([128, PW + 2 * SUB], FP8,
                                             name=f"ag_in{g}",
                                             tag=f"ag_in{g}")
                        nc.gpsimd.dma_start(
                            out=ag_in[:, 0:PW]
                                .rearrange("p (m v) -> p m v", m=MCH),
                            in_=s_ship[:, :, g * CW:(g + 1) * CW])
                        nc.gpsimd.dma_start(
                            out=ag_in[:, PW:PW + SUB],
                            in_=hi8[:, g * SUB:(g + 1) * SUB])
                        nc.gpsimd.dma_start(
                            out=ag_in[:, PW + SUB:],
                            in_=lo8[:, g * SUB:(g + 1) * SUB])
                        ag_out = cc_pool.tile(
                            [n_cores * 128, PW + 2 * SUB], FP8,
                            name=f"ag_out{g}", tag=f"ag_out{g}",
                            addr_space="Shared")
                        nc.gpsimd.collective_compute(
                            "AllGather", mybir.AluOpType.bypass,
                            replica_groups=[list(range(n_cores))],
                            ins=[ag_in], outs=[ag_out])
                        outs.append(ag_out)
                    return outs

                # Ship right away: every hop of the (fast) normalize chain
                # runs while the query encode keeps the PE busy, and the
                # AllGather start is gated by the LAST core's doorbell, so
                # early shipping matters more than avoiding the small
                # DVE/ACT head-of-line blocking.
                ag_outs = ship_support()

                # --- query encode: m-outer, stream both 512-blocks per
                # weight pair (one LDWEIGHTS per pair) ---
                pre_q = [pre_pool.tile([128, MCH, 512], BF16, tag="pre",
                                       name=f"pre_q{b}") for b in range(NB_Q)]
                sq_q = [sq_pool.tile([128, MCH, 512], FP8, tag="sq",
                                     name=f"sq_q{b}") for b in range(NB_Q)]
                n2_q = [ps_n2.tile([1, 512], F32, tag="n2", name=f"n2q{b}")
                        for b in range(NB_Q)]
                for m in range(MCH):
                    psq = [ps_mm.tile([128, 512], F32, tag="mmps", name="psq")
                           for _ in range(NB_Q)]
                    for t in range(KCH // 2):
                        for bq in range(NB_Q):
                            nc.tensor.matmul(
                                psq[bq], lhsT=W_sb[m][:, 2 * t:2 * t + 2, :],
                                rhs=q_xks[bq][:, 2 * t:2 * t + 2, :],
                                start=(t == 0), stop=(t == KCH // 2 - 1),
                                perf_mode=DR)
                    for bq in range(NB_Q):
                        nc.scalar.activation(pre_q[bq][:, m, :], psq[bq],
                                             AF.Identity,
                                             bias=b_sb[:, m:m + 1],
                                             scale=1.0 / W_SCALE)
                        nc.vector.tensor_mul(sq_q[bq][:, m, :],
                                             pre_q[bq][:, m, :],
                                             pre_q[bq][:, m, :])
                    if m >= 2 and m % 2 == 0:
                        p = m // 2 - 1
                        for bq in range(NB_Q):
                            nc.tensor.matmul(
                                n2_q[bq], lhsT=ones2[:, :, 0:1],
                                rhs=sq_q[bq][:, 2 * p:2 * p + 2, :],
                                start=(p == 0), stop=False, perf_mode=DR)
                p = MCH // 2 - 1
                for bq in range(NB_Q):
                    nc.tensor.matmul(n2_q[bq], lhsT=ones2[:, :, 0:1],
                                     rhs=sq_q[bq][:, 2 * p:2 * p + 2, :],
                                     start=False, stop=True, perf_mode=DR)
                for bq in range(NB_Q):
                    norm_chain(
                        n2_q[bq], pre_q[bq],
                        [q_nrm[:, m, bq * 512:(bq + 1) * 512]
                         for m in range(MCH)], 512)

                # --- gather reads: g-major, split across sync/gpsimd ---
                PW = MCH * CW
                for g in range(G):
                    for c in range(n_cores):
                        eng = nc.sync if c % 2 == 0 else nc.gpsimd
                        eng.dma_start(
                            out=gt[g][c],
                            in_=ag_outs[g][c * 128:(c + 1) * 128, 0:PW]
                                .rearrange("p (m v) -> p m v", m=MCH))
                        eng.dma_start(
                            out=gt_n2[g][c],
                            in_=ag_outs[g][c * 128:(c + 1) * 128, PW:])

            # --- sims + exp + preds (interleaved) ---
            with (
                tc.tile_pool(name="exp_pool", bufs=6) as exp_pool,
                tc.tile_pool(name="inv_pool", bufs=10) as inv_pool,
                tc.tile_pool(name="ps_pred", bufs=1, space="PSUM") as ps_pred,
                tc.tile_pool(name="fin", bufs=2) as fin,
            ):
                pp = [ps_pred.tile([64, 512], F32, tag=f"pp{qh}",
                                   name=f"pp{qh}") for qh in range(2)]
                work = [(c * (G * SUB) + g * SUB + i, g, c, i)
                        for g in range(G) for c in range(n_cores)
                        for i in range(SUB)]
                ebuf = {}
                NW = len(work)

                def preds_mm(idx):
                    sb = work[idx][0]
                    for qh in range(2):
                        nc.tensor.matmul(
                            pp[qh], lhsT=oh_sb[:, sb, :],
                            rhs=ebuf[sb][:, qh * 512:(qh + 1) * 512],
                            start=(idx == 0), stop=(idx == NW - 1))

                invsc = {}

                def inv_chain(g, c):
                    # rebuild 1/(256*||row||) for this core's chunk:
                    # n2 = 1024 + 16*(hi+lo); sqrt(65536*n2) = 256*sqrt(n2)
                    dsum = small.tile([128, SUB], F32, tag="dsum",
                                      name="dsum")
                    nc.vector.tensor_add(dsum,
                                         gt_n2[g][c][:, 0:SUB],
                                         gt_n2[g][c][:, SUB:2 * SUB])
                    n256 = small.tile([128, SUB], F32, tag="n256",
                                      name="n256")
                    nc.scalar.activation(n256, dsum, AF.Sqrt,
                                         scale=float(16 * 65536),
                                         bias=b2to26)
                    iv = inv_pool.tile([128, SUB], F32, tag="iv", name="iv")
                    nc.vector.reciprocal(iv, n256)
                    invsc[(g, c)] = iv

                for wi, (sb, g, c, i) in enumerate(work):
                    # prefetch the whole chunk's inv chains ahead of its sims
                    if (g, c) not in invsc:
                        for cc_ in range(c, n_cores):
                            inv_chain(g, cc_)
                    expb = exp_pool.tile([128, NQ], BF16, tag="expb",
                                         name="expb")
                    ebuf[sb] = expb
                    ss = slice(i * 128, (i + 1) * 128)
                    ps2 = [ps_mm.tile([128, 512], F32, tag="mmps", name="ps2")
                           for _ in range(2)]
                    for t in range(MCH // 2):
                        for qh in range(2):
                            nc.tensor.matmul(
                                ps2[qh],
                                lhsT=gt[g][c][:, 2 * t:2 * t + 2, ss],
                                rhs=q_nrm[:, 2 * t:2 * t + 2,
                                          qh * 512:(qh + 1) * 512],
                                start=(t == 0), stop=(t == MCH // 2 - 1),
                                perf_mode=DR)
                    for qh in range(2):
                        nc.scalar.activation(
                            expb[:, qh * 512:(qh + 1) * 512], ps2[qh], AF.Exp,
                            scale=invsc[(g, c)][:, i:i + 1])
                    if wi >= 2:
                        preds_mm(wi - 2)
                preds_mm(NW - 2)
                preds_mm(NW - 1)

                # --- drain class numerators; softmax divide happens on the
                # host (num / num.sum(axis=cls) is exactly the softmax) ---
                out_f = fin.tile([64, NQ], F32, tag="of", name="of")
                for qh in range(2):
                    nc.scalar.activation(out_f[:, qh * 512:(qh + 1) * 512],
                                         pp[qh], AF.Identity)
                nc.sync.dma_start(out=outd[:, :], in_=out_f)
    nc.finalize()
    return nc


_NC_CACHE = {}


def _get_nc(key):
    if key not in _NC_CACHE:
        NS, NQ, IN, EMB, NCLS = key
        _NC_CACHE[key] = build_nc(NS, NQ, IN, EMB, NCLS)
    return _NC_CACHE[key]


def _x_layout(x, kch, bs=512):
    """[NV, IN] fp32 -> [NV/bs, 128, KCH, bs] fp8:
    H[nb,p,k,v] = x[nb*bs+v, k*128+p]."""
    nv, in_dim = x.shape
    h = x.reshape(nv // bs, bs, kch, 128).transpose(0, 3, 2, 1)
    return np.ascontiguousarray(h.astype(ml_dtypes.float8_e4m3))


def _prep_inputs(support, query, W, b, support_labels, num_classes, n_cores):
    ncls = int(num_classes)
    bf = ml_dtypes.bfloat16
    support = np.asarray(support, np.float32)
    query = np.asarray(query, np.float32)
    W = np.asarray(W, np.float32)
    in_dim, emb = W.shape
    kch, mch = in_dim // 128, emb // 128
    ns = support.shape[0]
    Wh = np.ascontiguousarray(
        (W * W_SCALE).reshape(kch, 128, mch, 128)
        .transpose(2, 1, 0, 3).astype(ml_dtypes.float8_e4m3))
    bh = np.ascontiguousarray(np.asarray(b, np.float32).reshape(mch, 128).T)
    labels = np.asarray(support_labels).astype(np.int64)
    oh = np.zeros((ns, ncls), dtype=bf)
    oh[np.arange(ns), labels] = 1
    # oh[p, j, c] = onehot[j*128+p, c]  (j = global 128-row support chunk)
    ohh = np.ascontiguousarray(
        oh.reshape(ns // 128, 128, ncls).transpose(1, 0, 2))
    nq_shard = query.shape[0] // n_cores
    ns_shard = ns // n_cores
    qh_all = _x_layout(query, kch)
    nbq = nq_shard // 512
    in_maps = []
    for i in range(n_cores):
        sup_i = support[i * ns_shard:(i + 1) * ns_shard]
        in_maps.append({
            "supX": _x_layout(sup_i, kch)[0],
            "qX": np.ascontiguousarray(qh_all[i * nbq:(i + 1) * nbq]),
            "W": Wh,
            "b": bh,
            "onehot": ohh,
        })
    return in_maps


def _run(support, query, W, b, support_labels, num_classes, trace=False):
    ncls = int(num_classes)
    key = (support.shape[0], query.shape[0] // N_CORES, support.shape[1],
           W.shape[1], ncls)
    nc = _get_nc(key)
    in_maps = _prep_inputs(support, query, W, b, support_labels, ncls, N_CORES)
    res = run_bass_kernel_spmd(nc, in_maps, list(range(N_CORES)), trace=trace)
    # device output is [C, NQ_SHARD] f32 class numerators per core;
    # transpose + softmax-normalize + concat on host
    outs = []
    for r in res.results:
        num = np.asarray(r["out"], np.float32)      # [C, NQ_SHARD]
        outs.append((num / num.sum(axis=0, keepdims=True)).T)
    return np.concatenate(outs, axis=0).astype(np.float32), res


def kernel(support, query, W, b, support_labels, num_classes):
    out, _ = _run(support, query, W, b, support_labels, num_classes,
                  trace=False)
    return out
